# revision 14
# baseline (speedup 1.0000x reference)
"""DCM (dynamic conv module) Trainium2 kernel — single-pass x streaming.

Reference computation (per sample b, channel c):
  f[b,c,3,3]  = adaptive_avg_pool2d(x[b,c], 3)        # dynamic depthwise filter
  out[b,c]    = depthwise_conv3x3(x[b,c], f[b,c])     # zero padding 1
  y           = relu(batchnorm_train(out, gamma, beta))  # batch stats over (B,H,W)

Sharding: data-parallel over batch B=16 across 8 cores (2 samples/core).
Sync-BN via a [C,2] AllReduce of per-channel (sum, sumsq).

Per-core layout: channels C=128 on partitions.  x is streamed from HBM
ONCE per sample as 8 halo chunks of 18 rows, stored ROW-PADDED in SBUF
as [C, 18, W+2] with zeroed pad columns, so all 9 conv taps read
in-bounds zeros at the horizontal edges — no wrap fixups.  Tiles are
declared float32r (PE runs 1 cyc/row; DMA + DVE use bitcast views).

Conv per half-chunk (8 output rows, [C,1024] PSUM tile, 4 PSUM bufs):
DVE first writes the two dj==0 taps into PSUM (mul + scalar_tensor_
tensor), then the PE accumulates the other 7 taps on top with
start=False matmuls against diag(f_tap) stationaries.  ACT copies
PSUM->bf16 SBUF (fused per-channel sum accum) and squares in place
(fused sumsq), keeping BN statistics exact in f32.

Sample-0 pooling runs on DVE during the load; sample-1 pooling is
split DVE/gpsimd under sample-0's conv (gpsimd is blocked early by the
warm-up AllReduce that absorbs the ~40us ncfw init barrier).  After
the stats AllReduce, BN+ReLU is applied in-place (ACT/DVE split) and y
is written back as bf16 (converted to f32 on the host).
"""

import numpy as np

# ---------------------------------------------------------------- constants
B, C, H, W = 16, 128, 128, 128
N_CORES = 8
BL = B // N_CORES          # samples per core
HW = H * W                 # 16384 free elems per plane
FS = 3
BN_EPS = 1e-5

ROWS = 16                  # x rows per chunk
NCHUNK = H // ROWS         # 8 chunks per plane
WP = W + 2                 # padded row width
XROWS = ROWS + 2           # halo rows per chunk
XT_F = XROWS * WP          # 2340 padded elems per x tile

PROWS = 8                  # output rows per psum tile
P_F = PROWS * W            # 1024 free elems per psum tile
HALVES = ROWS // PROWS     # psum tiles per x chunk
NPS = NCHUNK * BL * HALVES  # psum/out tiles per core (32)

# adaptive_avg_pool2d(3) bin boundaries (PyTorch convention)
SH = [(i * H) // FS for i in range(FS)]
EH = [-((-(i + 1) * H) // FS) for i in range(FS)]
SW = [(i * W) // FS for i in range(FS)]
EW = [-((-(i + 1) * W) // FS) for i in range(FS)]

TAPS = [(di, dj) for di in (-1, 0, 1) for dj in (-1, 0, 1)]  # t = 3*(di+1)+(dj+1)
DVE_TAPS = [1, 7]                      # (di=-1,dj=0), (di=+1,dj=0)
PE_TAPS = [t for t in range(FS * FS) if t not in DVE_TAPS]

MM_ROWS = 4                # output rows per matmul (512 moving elems)
MM_N = MM_ROWS * W
NSL = PROWS // MM_ROWS     # matmul slices per psum tile

# BN apply split: of every 8 out tiles, this many go to ACT (1 op each),
# the rest to DVE (2 ops each, bf16 in-place).
ACT_BN_PER8 = 4


def _counts_recip():
    cr = np.empty((C, FS * FS), dtype=np.float32)
    for i in range(FS):
        for j in range(FS):
            cr[:, 3 * i + j] = 1.0 / float((EH[i] - SH[i]) * (EW[j] - SW[j]))
    return cr


def build_nc(n_cores: int = N_CORES):
    """Build + compile the per-core Bass program (identical on all cores)."""
    import concourse.bacc as bacc
    import concourse.tile as tile
    from concourse import mybir

    f32 = mybir.dt.float32
    f32r = mybir.dt.float32r
    bf16 = mybir.dt.bfloat16
    AT = mybir.ActivationFunctionType
    OP = mybir.AluOpType
    AX = mybir.AxisListType

    ntot = float(n_cores * BL * HW)   # BN element count per channel

    nc = bacc.Bacc(
        "TRN2",
        target_bir_lowering=False,
        debug=False,
        num_devices=n_cores,
    )

    x_d = nc.dram_tensor("x", [BL, C, HW], f32, kind="ExternalInput").ap()
    gamma_d = nc.dram_tensor("gamma", [C, 1], f32, kind="ExternalInput").ap()
    beta_d = nc.dram_tensor("beta", [C, 1], f32, kind="ExternalInput").ap()
    ident_d = nc.dram_tensor("ident", [C, C], f32, kind="ExternalInput").ap()
    crecip_d = nc.dram_tensor("crecip", [C, FS * FS], f32, kind="ExternalInput").ap()
    y_d = nc.dram_tensor("y", [BL, C, HW], bf16, kind="ExternalOutput").ap()

    with tile.TileContext(nc) as tc:
        with (
            tc.tile_pool(name="singles", bufs=1) as singles,
            tc.tile_pool(name="xpool", bufs=10) as xpool,
            tc.tile_pool(name="outres", bufs=NPS) as outres,
            tc.tile_pool(name="psum", bufs=4, space="PSUM") as psum,
            tc.tile_pool(name="colsp", bufs=2) as colsp,
            tc.tile_pool(name="fpool", bufs=2) as fpool,
            tc.tile_pool(name="diagp", bufs=2 * len(PE_TAPS)) as diagp,
            tc.tile_pool(name="statp", bufs=1) as statp,
            tc.tile_pool(name="dram", bufs=1, space="DRAM") as dram,
        ):
            # Dummy warm-up AllReduce issued immediately (input is garbage
            # DRAM contents — only the ncfw init barrier + ramp matter).
            # The real stats AllReduce later is then cheap, and gpsimd's
            # in-order queue is only blocked early in the run.
            dw_in = dram.tile([C, 2], f32, tag="dw_in")
            dw_out = dram.tile([C, 2], f32, tag="dw_out")
            nc.gpsimd.collective_compute(
                "AllReduce",
                OP.add,
                replica_groups=[list(range(n_cores))],
                ins=[dw_in[:].opt()],
                outs=[dw_out[:].opt()],
            )

            # ---- constants
            gamma_s = singles.tile([C, 1], f32, tag="gamma")
            nc.sync.dma_start(out=gamma_s[:], in_=gamma_d[:, :])
            beta_s = singles.tile([C, 1], f32, tag="beta")
            nc.sync.dma_start(out=beta_s[:], in_=beta_d[:, :])
            ident_s = singles.tile([C, C], f32, tag="ident")
            nc.sync.dma_start(out=ident_s[:], in_=ident_d[:, :])
            crecip_s = singles.tile([C, FS * FS], f32, tag="crecip")
            nc.sync.dma_start(out=crecip_s[:], in_=crecip_d[:, :])

            sums = statp.tile([C, NPS], f32, tag="sums")
            sumsq = statp.tile([C, NPS], f32, tag="sumsq")

            out_tiles = []
            kpt = 0  # global psum-tile index

            # per-sample state
            xts = {}      # (s, c) -> (f32r 3d view, f32 3d view)
            colS = {}     # s -> column-sum tile
            fTs = {}      # s -> filter tile [C, 9] f32
            diags = {}    # s -> {t: diag tile}

            def emit_load(s, c, dma_engine):
                """DMA one halo chunk of plane s into a row-padded x tile."""
                xt = xpool.tile([C, XT_F], f32r, tag="xt", name="xt")
                x3r = xt[:].rearrange("p (r w) -> p r w", w=WP)
                x3f = xt[:].bitcast(f32).rearrange("p (r w) -> p r w", w=WP)
                # pad columns must be zero so dj=+-1 taps read zeros
                nc.vector.memset(x3f[:, :, 0:1], 0.0)
                nc.vector.memset(x3f[:, :, WP - 1:WP], 0.0)
                r_lo = c * ROWS - 1
                r_hi = c * ROWS + ROWS + 1
                if r_lo < 0:
                    nc.vector.memset(x3f[:, 0:1, :], 0.0)
                if r_hi > H:
                    nc.vector.memset(x3f[:, XROWS - 1:XROWS, :], 0.0)
                src_lo = max(r_lo, 0)
                src_hi = min(r_hi, H)
                dst_lo = src_lo - r_lo
                dma_engine.dma_start(
                    out=x3r[:, dst_lo:dst_lo + (src_hi - src_lo), 1:1 + W],
                    in_=x_d[s, :, src_lo * W:src_hi * W].bitcast(f32r),
                )
                xts[(s, c)] = (x3r, x3f)

            def emit_pool(s, c, eng):
                if s not in colS:
                    colS[s] = colsp.tile([C, FS, H], f32, tag="colS", name="colS")
                _, x3f = xts[(s, c)]
                for j in range(FS):
                    eng.tensor_reduce(
                        out=colS[s][:, j, c * ROWS:(c + 1) * ROWS],
                        in_=x3f[:, 1:1 + ROWS, 1 + SW[j]:1 + EW[j]],
                        axis=AX.X,
                        op=OP.add,
                    )

            def emit_filter(s):
                fT = fpool.tile([C, FS * FS], f32, tag="fT", name="fT")
                for i in range(FS):
                    for j in range(FS):
                        k = 3 * i + j
                        nc.vector.tensor_reduce(
                            out=fT[:, k:k + 1],
                            in_=colS[s][:, j, SH[i]:EH[i]],
                            axis=AX.X,
                            op=OP.add,
                        )
                nc.vector.tensor_mul(fT[:], fT[:], crecip_s[:])
                dgs = {}
                for t in PE_TAPS:
                    dg = diagp.tile([C, C], f32r, tag="diag", name="diag")
                    nc.vector.tensor_scalar_mul(dg[:], ident_s[:], fT[:, t:t + 1])
                    dgs[t] = dg
                fTs[s], diags[s] = fT, dgs

            def emit_conv(s, c):
                """Both psum halves of chunk (s, c)."""
                nonlocal kpt
                x3r, x3f = xts.pop((s, c))
                fT, dgs = fTs[s], diags[s]
                for h in range(HALVES):
                    pt = psum.tile([C, P_F], f32, tag="pt", name="pt")
                    p3 = pt[:].rearrange("p (r w) -> p r w", w=W)
                    # PE accumulates the 7 dj!=0 + center taps
                    for sl in range(NSL):
                        for it, t in enumerate(PE_TAPS):
                            di, dj = TAPS[t]
                            rlo = h * PROWS + sl * MM_ROWS + 1 + di
                            nc.tensor.matmul(
                                pt[:, sl * MM_N:(sl + 1) * MM_N],
                                dgs[t][:],
                                x3r[:, rlo:rlo + MM_ROWS, 1 + dj:1 + dj + W],
                                start=(it == 0),
                                stop=(it == len(PE_TAPS) - 1),
                            )
                    # DVE adds the two dj==0 taps into PSUM (RMW)
                    for t in DVE_TAPS:
                        di, _ = TAPS[t]
                        r0 = h * PROWS + 1 + di
                        nc.vector.scalar_tensor_tensor(
                            out=p3[:, :, :],
                            in0=x3f[:, r0:r0 + PROWS, 1:1 + W],
                            scalar=fT[:, t:t + 1],
                            in1=p3[:, :, :],
                            op0=OP.mult,
                            op1=OP.add,
                        )
                    # PSUM -> resident bf16 SBUF copy, fused per-channel sum
                    ot = outres.tile([C, P_F], bf16, tag="ot", name="ot")
                    nc.scalar.activation(
                        out=ot[:], in_=pt[:], func=AT.Copy,
                        accum_out=sums[:, kpt:kpt + 1],
                    )
                    # sum of squares from the exact f32 psum values
                    nc.scalar.activation(
                        out=pt[:], in_=pt[:], func=AT.Square,
                        accum_out=sumsq[:, kpt:kpt + 1],
                    )
                    out_tiles.append((s, c, h, ot))
                    kpt += 1

            # ---------------- software-pipelined emission
            for c in range(NCHUNK):
                # alternate HWDGE queues (SP / Activation) for the first load
                emit_load(0, c, nc.sync if c % 2 == 0 else nc.scalar)
                emit_pool(0, c, nc.vector)
            emit_load(1, 0, nc.sync)
            emit_load(1, 1, nc.sync)
            emit_filter(0)
            for c in range(NCHUNK):
                if c + 2 < NCHUNK:
                    emit_load(1, c + 2, nc.sync)
                # sample-1 pooling interleaved with sample-0 conv on DVE;
                # its filter/diag build lands under sample-0's last chunk
                emit_pool(1, c, nc.vector)
                if c == NCHUNK - 1:
                    emit_filter(1)
                emit_conv(0, c)
            for c in range(NCHUNK):
                emit_conv(1, c)

            # ---------------- sync-BN stats AllReduce
            arin = statp.tile([C, 2], f32, tag="arin")
            nc.vector.tensor_reduce(out=arin[:, 0:1], in_=sums[:], axis=AX.X, op=OP.add)
            nc.vector.tensor_reduce(out=arin[:, 1:2], in_=sumsq[:], axis=AX.X, op=OP.add)
            d_in = dram.tile([C, 2], f32, tag="d_in")
            d_out = dram.tile([C, 2], f32, tag="d_out")
            nc.sync.dma_start(out=d_in[:], in_=arin[:])
            nc.gpsimd.collective_compute(
                "AllReduce",
                OP.add,
                replica_groups=[list(range(n_cores))],
                ins=[d_in[:].opt()],
                outs=[d_out[:].opt()],
            )
            aro = statp.tile([C, 2], f32, tag="aro")
            nc.sync.dma_start(out=aro[:], in_=d_out[:])

            # ---------------- BN scale/shift (all [C,1], fp32)
            mean = statp.tile([C, 1], f32, tag="mean")
            nc.vector.tensor_scalar_mul(mean[:], aro[:, 0:1], 1.0 / ntot)
            ex2 = statp.tile([C, 1], f32, tag="ex2")
            nc.vector.tensor_scalar_mul(ex2[:], aro[:, 1:2], 1.0 / ntot)
            var = statp.tile([C, 1], f32, tag="var")
            nc.vector.tensor_mul(var[:], mean[:], mean[:])
            nc.vector.tensor_sub(var[:], ex2[:], var[:])
            veps = statp.tile([C, 1], f32, tag="veps")
            nc.vector.tensor_scalar_add(veps[:], var[:], BN_EPS)
            eps_t = statp.tile([C, 1], f32, tag="eps_t")
            nc.vector.memset(eps_t[:], BN_EPS)
            sd = statp.tile([C, 1], f32, tag="sd")
            nc.scalar.activation(out=sd[:], in_=var[:], func=AT.Sqrt, bias=eps_t[:])
            z = statp.tile([C, 1], f32, tag="z")
            nc.vector.reciprocal(z[:], sd[:])
            # one Newton step: z <- z * (1.5 - 0.5 * veps * z^2)
            nt = statp.tile([C, 1], f32, tag="nt")
            nc.vector.tensor_mul(nt[:], z[:], z[:])
            nc.vector.tensor_mul(nt[:], nt[:], veps[:])
            nc.vector.tensor_scalar(
                out=nt[:], in0=nt[:], scalar1=-0.5, scalar2=1.5,
                op0=OP.mult, op1=OP.add,
            )
            nc.vector.tensor_mul(z[:], z[:], nt[:])
            scale_t = statp.tile([C, 1], f32, tag="scale_t")
            nc.vector.tensor_mul(scale_t[:], gamma_s[:], z[:])
            shift_t = statp.tile([C, 1], f32, tag="shift_t")
            nc.vector.tensor_mul(shift_t[:], mean[:], scale_t[:])
            nc.vector.tensor_sub(shift_t[:], beta_s[:], shift_t[:])

            # ---------------- BN apply + ReLU + writeback (ACT / DVE split)
            for idx, (s, c, h, ot) in enumerate(out_tiles):
                if idx % 8 < ACT_BN_PER8:
                    nc.scalar.activation(
                        out=ot[:], in_=ot[:], func=AT.Relu,
                        scale=scale_t[:], bias=shift_t[:],
                    )
                else:
                    nc.vector.tensor_scalar(
                        out=ot[:], in0=ot[:],
                        scalar1=scale_t[:], scalar2=shift_t[:],
                        op0=OP.mult, op1=OP.add,
                    )
                    nc.vector.tensor_scalar_max(ot[:], ot[:], 0.0)
                off = c * (ROWS * W) + h * P_F
                nc.sync.dma_start(
                    out=y_d[s, :, off:off + P_F], in_=ot[:],
                )

    nc.compile()
    return nc


_NC_CACHE = {}


def _get_nc(n_cores: int = N_CORES):
    if n_cores not in _NC_CACHE:
        _NC_CACHE[n_cores] = build_nc(n_cores)
    return _NC_CACHE[n_cores]


def make_in_maps(x: np.ndarray, gamma: np.ndarray, beta: np.ndarray,
                 n_cores: int = N_CORES):
    x_r = np.ascontiguousarray(
        np.asarray(x, dtype=np.float32).reshape(B, C, HW)
    )
    g = np.ascontiguousarray(np.asarray(gamma, dtype=np.float32).reshape(C, 1))
    b = np.ascontiguousarray(np.asarray(beta, dtype=np.float32).reshape(C, 1))
    ident = np.eye(C, dtype=np.float32)
    crecip = _counts_recip()
    maps = []
    for core in range(n_cores):
        maps.append({
            "x": x_r[core * BL:(core + 1) * BL],
            "gamma": g,
            "beta": b,
            "ident": ident,
            "crecip": crecip,
        })
    return maps


def kernel(x, gamma, beta):
    from concourse import bass_utils

    nc = _get_nc(N_CORES)
    in_maps = make_in_maps(x, gamma, beta, N_CORES)
    res = bass_utils.run_bass_kernel_spmd(nc, in_maps, core_ids=list(range(N_CORES)))
    y = np.concatenate(
        [np.asarray(res.results[c]["y"]) for c in range(N_CORES)], axis=0
    )
    return y.reshape(B, C, H, W).astype(np.float32)


# revision 20
# speedup vs baseline: 1.3733x; 1.3733x over previous
"""DCM (dynamic conv module) Trainium2 kernel — single-pass x streaming.

Reference computation (per sample b, channel c):
  f[b,c,3,3]  = adaptive_avg_pool2d(x[b,c], 3)        # dynamic depthwise filter
  out[b,c]    = depthwise_conv3x3(x[b,c], f[b,c])     # zero padding 1
  y           = relu(batchnorm_train(out, gamma, beta))  # batch stats over (B,H,W)

Sharding: data-parallel over batch B=16 across 8 cores (2 samples/core).
Sync-BN via a [C,2] AllReduce of per-channel (sum, sumsq).

Per-core layout: channels C=128 on partitions.  x is streamed from HBM
ONCE per sample as 8 halo chunks of 18 rows, stored ROW-PADDED in SBUF
as [C, 18, W+2] with zeroed pad columns, so all 9 conv taps read
in-bounds zeros at the horizontal edges — no wrap fixups.  Tiles are
declared float32r (PE runs 1 cyc/row; DMA + DVE use bitcast views).

Conv per half-chunk (8 output rows, [C,1024] PSUM tile, 4 PSUM bufs):
DVE first writes the two dj==0 taps into PSUM (mul + scalar_tensor_
tensor), then the PE accumulates the other 7 taps on top with
start=False matmuls against diag(f_tap) stationaries.  ACT copies
PSUM->bf16 SBUF (fused per-channel sum accum) and squares in place
(fused sumsq), keeping BN statistics exact in f32.

Sample-0 pooling runs on DVE during the load; sample-1 pooling is
split DVE/gpsimd under sample-0's conv (gpsimd is blocked early by the
warm-up AllReduce that absorbs the ~40us ncfw init barrier).  After
the stats AllReduce, BN+ReLU is applied in-place (ACT/DVE split) and y
is written back as bf16 (converted to f32 on the host).
"""

import numpy as np

# ---------------------------------------------------------------- constants
B, C, H, W = 16, 128, 128, 128
N_CORES = 8
BL = B // N_CORES          # samples per core
HW = H * W                 # 16384 free elems per plane
FS = 3
BN_EPS = 1e-5

ROWS = 16                  # x rows per chunk
NCHUNK = H // ROWS         # 8 chunks per plane
WP = W + 2                 # padded row width
XROWS = ROWS + 2           # halo rows per chunk
XT_F = XROWS * WP          # 2340 padded elems per x tile

PROWS = 8                  # output rows per psum tile
P_F = PROWS * W            # 1024 free elems per psum tile
HALVES = ROWS // PROWS     # psum tiles per x chunk
NPS = NCHUNK * BL * HALVES  # psum/out tiles per core (32)

# adaptive_avg_pool2d(3) bin boundaries (PyTorch convention)
SH = [(i * H) // FS for i in range(FS)]
EH = [-((-(i + 1) * H) // FS) for i in range(FS)]
SW = [(i * W) // FS for i in range(FS)]
EW = [-((-(i + 1) * W) // FS) for i in range(FS)]

TAPS = [(di, dj) for di in (-1, 0, 1) for dj in (-1, 0, 1)]  # t = 3*(di+1)+(dj+1)
DVE_TAPS = [1, 7]                      # (di=-1,dj=0), (di=+1,dj=0)
PE_TAPS = [t for t in range(FS * FS) if t not in DVE_TAPS]

MM_ROWS = 4                # output rows per matmul (512 moving elems)
MM_N = MM_ROWS * W
NSL = PROWS // MM_ROWS     # matmul slices per psum tile

# BN apply split: of every 8 out tiles, this many go to ACT (1 op each),
# the rest to DVE (2 ops each, bf16 in-place).
ACT_BN_PER8 = 4


def _counts_recip():
    cr = np.empty((C, FS * FS), dtype=np.float32)
    for i in range(FS):
        for j in range(FS):
            cr[:, 3 * i + j] = 1.0 / float((EH[i] - SH[i]) * (EW[j] - SW[j]))
    return cr


def build_nc(n_cores: int = N_CORES):
    """Build + compile the per-core Bass program (identical on all cores)."""
    import concourse.bacc as bacc
    import concourse.tile as tile
    from concourse import mybir

    f32 = mybir.dt.float32
    f32r = mybir.dt.float32r
    bf16 = mybir.dt.bfloat16
    AT = mybir.ActivationFunctionType
    OP = mybir.AluOpType
    AX = mybir.AxisListType

    ntot = float(n_cores * BL * HW)   # BN element count per channel

    nc = bacc.Bacc(
        "TRN2",
        target_bir_lowering=False,
        debug=False,
        num_devices=n_cores,
    )

    # x arrives pre-padded from the host: [H+2, W+2] planes with a zero
    # border, so chunk loads are single contiguous DMAs and all 9 conv
    # taps read in-bounds zeros at the edges.
    x_d = nc.dram_tensor(
        "x", [BL, C, (H + 2) * WP], f32, kind="ExternalInput"
    ).ap()
    gamma_d = nc.dram_tensor("gamma", [C, 1], f32, kind="ExternalInput").ap()
    beta_d = nc.dram_tensor("beta", [C, 1], f32, kind="ExternalInput").ap()
    ident_d = nc.dram_tensor("ident", [C, C], f32, kind="ExternalInput").ap()
    crecip_d = nc.dram_tensor("crecip", [C, FS * FS], f32, kind="ExternalInput").ap()
    y_d = nc.dram_tensor("y", [BL, C, HW], bf16, kind="ExternalOutput").ap()

    with tile.TileContext(nc) as tc:
        with (
            tc.tile_pool(name="singles", bufs=1) as singles,
            tc.tile_pool(name="xpool", bufs=12) as xpool,
            tc.tile_pool(name="outres", bufs=NPS) as outres,
            tc.tile_pool(name="psum", bufs=4, space="PSUM") as psum,
            tc.tile_pool(name="colsp", bufs=2) as colsp,
            tc.tile_pool(name="fpool", bufs=2) as fpool,
            tc.tile_pool(name="diagp", bufs=2 * len(PE_TAPS)) as diagp,
            tc.tile_pool(name="statp", bufs=1) as statp,
            tc.tile_pool(name="dram", bufs=1, space="DRAM") as dram,
        ):
            # Dummy warm-up AllReduce issued immediately (input is garbage
            # DRAM contents — only the ncfw init barrier + ramp matter).
            # The real stats AllReduce later is then cheap, and gpsimd's
            # in-order queue is only blocked early in the run.
            dw_in = dram.tile([C, 2], f32, tag="dw_in")
            dw_out = dram.tile([C, 2], f32, tag="dw_out")
            nc.gpsimd.collective_compute(
                "AllReduce",
                OP.add,
                replica_groups=[list(range(n_cores))],
                ins=[dw_in[:].opt()],
                outs=[dw_out[:].opt()],
            )

            # ---- constants
            gamma_s = singles.tile([C, 1], f32, tag="gamma")
            nc.sync.dma_start(out=gamma_s[:], in_=gamma_d[:, :])
            beta_s = singles.tile([C, 1], f32, tag="beta")
            nc.sync.dma_start(out=beta_s[:], in_=beta_d[:, :])
            ident_s = singles.tile([C, C], f32, tag="ident")
            nc.sync.dma_start(out=ident_s[:], in_=ident_d[:, :])
            crecip_s = singles.tile([C, FS * FS], f32, tag="crecip")
            nc.sync.dma_start(out=crecip_s[:], in_=crecip_d[:, :])

            sums = statp.tile([C, NPS], f32, tag="sums")
            sumsq = statp.tile([C, NPS], f32, tag="sumsq")

            out_tiles = []
            kpt = 0  # global psum-tile index

            # per-sample state
            xts = {}      # (s, c) -> (f32r 3d view, f32 3d view)
            colS = {}     # s -> column-sum tile
            fTs = {}      # s -> filter tile [C, 9] f32
            diags = {}    # s -> {t: diag tile}

            def emit_load(s, c, dma_engine):
                """DMA one halo chunk of plane s (contiguous, pre-padded)."""
                xt = xpool.tile([C, XT_F], f32r, tag="xt", name="xt")
                x3r = xt[:].rearrange("p (r w) -> p r w", w=WP)
                x3f = xt[:].bitcast(f32).rearrange("p (r w) -> p r w", w=WP)
                # padded row (c*16) is image row c*16-1; 18 rows cover the halo
                src = c * ROWS * WP
                dma_engine.dma_start(
                    out=xt[:],
                    in_=x_d[s, :, src:src + XT_F].bitcast(f32r),
                )
                xts[(s, c)] = (x3r, x3f)

            def emit_pool(s, c, eng):
                if s not in colS:
                    colS[s] = colsp.tile([C, FS, H], f32, tag="colS", name="colS")
                _, x3f = xts[(s, c)]
                for j in range(FS):
                    eng.tensor_reduce(
                        out=colS[s][:, j, c * ROWS:(c + 1) * ROWS],
                        in_=x3f[:, 1:1 + ROWS, 1 + SW[j]:1 + EW[j]],
                        axis=AX.X,
                        op=OP.add,
                    )

            def emit_filter(s):
                fT = fpool.tile([C, FS * FS], f32, tag="fT", name="fT")
                for i in range(FS):
                    for j in range(FS):
                        k = 3 * i + j
                        nc.vector.tensor_reduce(
                            out=fT[:, k:k + 1],
                            in_=colS[s][:, j, SH[i]:EH[i]],
                            axis=AX.X,
                            op=OP.add,
                        )
                nc.vector.tensor_mul(fT[:], fT[:], crecip_s[:])
                dgs = {}
                for t in PE_TAPS:
                    dg = diagp.tile([C, C], f32r, tag="diag", name="diag")
                    nc.vector.tensor_scalar_mul(dg[:], ident_s[:], fT[:, t:t + 1])
                    dgs[t] = dg
                fTs[s], diags[s] = fT, dgs

            def emit_conv(s, c):
                """Both psum halves of chunk (s, c)."""
                nonlocal kpt
                x3r, x3f = xts.pop((s, c))
                fT, dgs = fTs[s], diags[s]
                for h in range(HALVES):
                    pt = psum.tile([C, P_F], f32, tag="pt", name="pt")
                    p3 = pt[:].rearrange("p (r w) -> p r w", w=W)
                    # PE accumulates the 7 dj!=0 + center taps
                    for sl in range(NSL):
                        for it, t in enumerate(PE_TAPS):
                            di, dj = TAPS[t]
                            rlo = h * PROWS + sl * MM_ROWS + 1 + di
                            nc.tensor.matmul(
                                pt[:, sl * MM_N:(sl + 1) * MM_N],
                                dgs[t][:],
                                x3r[:, rlo:rlo + MM_ROWS, 1 + dj:1 + dj + W],
                                start=(it == 0),
                                stop=(it == len(PE_TAPS) - 1),
                            )
                    # DVE adds the two dj==0 taps into PSUM (RMW)
                    for t in DVE_TAPS:
                        di, _ = TAPS[t]
                        r0 = h * PROWS + 1 + di
                        nc.vector.scalar_tensor_tensor(
                            out=p3[:, :, :],
                            in0=x3f[:, r0:r0 + PROWS, 1:1 + W],
                            scalar=fT[:, t:t + 1],
                            in1=p3[:, :, :],
                            op0=OP.mult,
                            op1=OP.add,
                        )
                    # PSUM -> resident bf16 SBUF copy, fused per-channel sum
                    ot = outres.tile([C, P_F], bf16, tag="ot", name="ot")
                    nc.scalar.activation(
                        out=ot[:], in_=pt[:], func=AT.Copy,
                        accum_out=sums[:, kpt:kpt + 1],
                    )
                    # sum of squares from the exact f32 psum values
                    nc.scalar.activation(
                        out=pt[:], in_=pt[:], func=AT.Square,
                        accum_out=sumsq[:, kpt:kpt + 1],
                    )
                    out_tiles.append((s, c, h, ot))
                    kpt += 1

            # ---------------- software-pipelined emission
            for c in range(NCHUNK):
                # alternate HWDGE queues (SP / Activation) for the first load
                emit_load(0, c, nc.sync if c % 2 == 0 else nc.scalar)
                emit_pool(0, c, nc.vector)
            emit_load(1, 0, nc.sync)
            emit_load(1, 1, nc.sync)
            emit_filter(0)
            for c in range(NCHUNK):
                if c + 2 < NCHUNK:
                    emit_load(1, c + 2, nc.sync)
                # sample-1 pooling interleaved with sample-0 conv on DVE;
                # its filter/diag build lands under sample-0's last chunk
                emit_pool(1, c, nc.vector)
                if c == NCHUNK - 1:
                    emit_filter(1)
                emit_conv(0, c)
            for c in range(NCHUNK):
                emit_conv(1, c)

            # ---------------- sync-BN stats AllReduce
            arin = statp.tile([C, 2], f32, tag="arin")
            nc.vector.tensor_reduce(out=arin[:, 0:1], in_=sums[:], axis=AX.X, op=OP.add)
            nc.vector.tensor_reduce(out=arin[:, 1:2], in_=sumsq[:], axis=AX.X, op=OP.add)
            d_in = dram.tile([C, 2], f32, tag="d_in")
            d_out = dram.tile([C, 2], f32, tag="d_out")
            nc.sync.dma_start(out=d_in[:], in_=arin[:])
            nc.gpsimd.collective_compute(
                "AllReduce",
                OP.add,
                replica_groups=[list(range(n_cores))],
                ins=[d_in[:].opt()],
                outs=[d_out[:].opt()],
            )
            aro = statp.tile([C, 2], f32, tag="aro")
            nc.sync.dma_start(out=aro[:], in_=d_out[:])

            # ---------------- BN scale/shift (all [C,1], fp32)
            mean = statp.tile([C, 1], f32, tag="mean")
            nc.vector.tensor_scalar_mul(mean[:], aro[:, 0:1], 1.0 / ntot)
            ex2 = statp.tile([C, 1], f32, tag="ex2")
            nc.vector.tensor_scalar_mul(ex2[:], aro[:, 1:2], 1.0 / ntot)
            var = statp.tile([C, 1], f32, tag="var")
            nc.vector.tensor_mul(var[:], mean[:], mean[:])
            nc.vector.tensor_sub(var[:], ex2[:], var[:])
            veps = statp.tile([C, 1], f32, tag="veps")
            nc.vector.tensor_scalar_add(veps[:], var[:], BN_EPS)
            eps_t = statp.tile([C, 1], f32, tag="eps_t")
            nc.vector.memset(eps_t[:], BN_EPS)
            sd = statp.tile([C, 1], f32, tag="sd")
            nc.scalar.activation(out=sd[:], in_=var[:], func=AT.Sqrt, bias=eps_t[:])
            z = statp.tile([C, 1], f32, tag="z")
            nc.vector.reciprocal(z[:], sd[:])
            # one Newton step: z <- z * (1.5 - 0.5 * veps * z^2)
            nt = statp.tile([C, 1], f32, tag="nt")
            nc.vector.tensor_mul(nt[:], z[:], z[:])
            nc.vector.tensor_mul(nt[:], nt[:], veps[:])
            nc.vector.tensor_scalar(
                out=nt[:], in0=nt[:], scalar1=-0.5, scalar2=1.5,
                op0=OP.mult, op1=OP.add,
            )
            nc.vector.tensor_mul(z[:], z[:], nt[:])
            scale_t = statp.tile([C, 1], f32, tag="scale_t")
            nc.vector.tensor_mul(scale_t[:], gamma_s[:], z[:])
            shift_t = statp.tile([C, 1], f32, tag="shift_t")
            nc.vector.tensor_mul(shift_t[:], mean[:], scale_t[:])
            nc.vector.tensor_sub(shift_t[:], beta_s[:], shift_t[:])

            # ---------------- BN apply + ReLU + writeback (ACT / DVE split)
            for idx, (s, c, h, ot) in enumerate(out_tiles):
                if idx % 8 < ACT_BN_PER8:
                    nc.scalar.activation(
                        out=ot[:], in_=ot[:], func=AT.Relu,
                        scale=scale_t[:], bias=shift_t[:],
                    )
                else:
                    nc.vector.tensor_scalar(
                        out=ot[:], in0=ot[:],
                        scalar1=scale_t[:], scalar2=shift_t[:],
                        op0=OP.mult, op1=OP.add,
                    )
                    nc.vector.tensor_scalar_max(ot[:], ot[:], 0.0)
                off = c * (ROWS * W) + h * P_F
                nc.sync.dma_start(
                    out=y_d[s, :, off:off + P_F], in_=ot[:],
                )

    nc.compile()
    return nc


_NC_CACHE = {}


def _get_nc(n_cores: int = N_CORES):
    if n_cores not in _NC_CACHE:
        _NC_CACHE[n_cores] = build_nc(n_cores)
    return _NC_CACHE[n_cores]


def make_in_maps(x: np.ndarray, gamma: np.ndarray, beta: np.ndarray,
                 n_cores: int = N_CORES):
    xp = np.zeros((B, C, H + 2, W + 2), dtype=np.float32)
    xp[:, :, 1:H + 1, 1:W + 1] = np.asarray(x, dtype=np.float32)
    x_r = np.ascontiguousarray(xp.reshape(B, C, (H + 2) * (W + 2)))
    g = np.ascontiguousarray(np.asarray(gamma, dtype=np.float32).reshape(C, 1))
    b = np.ascontiguousarray(np.asarray(beta, dtype=np.float32).reshape(C, 1))
    ident = np.eye(C, dtype=np.float32)
    crecip = _counts_recip()
    maps = []
    for core in range(n_cores):
        maps.append({
            "x": x_r[core * BL:(core + 1) * BL],
            "gamma": g,
            "beta": b,
            "ident": ident,
            "crecip": crecip,
        })
    return maps


def kernel(x, gamma, beta):
    from concourse import bass_utils

    nc = _get_nc(N_CORES)
    in_maps = make_in_maps(x, gamma, beta, N_CORES)
    res = bass_utils.run_bass_kernel_spmd(nc, in_maps, core_ids=list(range(N_CORES)))
    y = np.concatenate(
        [np.asarray(res.results[c]["y"]) for c in range(N_CORES)], axis=0
    )
    return y.reshape(B, C, H, W).astype(np.float32)
